# revision 3
# baseline (speedup 1.0000x reference)
"""Trainium2 Bass kernel for nn_CritiGraph (ct_val expansion), 16-bit pipeline.

Math: ct_val[b,t1,t2,m,tp] = (dis_sum - dis_sta_pos + dis_cnc_pos)/TP with
dis(c1,c2,norm) = sign(c1)sign(c2) * (1 - table[|c1|^|c2|]) * norm and
table[x] = (floor(log2(x+1))+1)/16.  With X = |cnc|^|pos| = base ^ fm
(base[t2,tp] = ori^|pos|, fm[m,tp] the candidate xor-delta) and
e = floor(log2(X+1)):  out = scale_run*(e+15) + bias_run, where
scale_run = -sgn*ps*valn/(H*TP) and bias_run = A/TP - 30*scale_run are
per-(tok,t2,tp) scalars (sgn structural per m-run).

Device pipeline per [128, L] chunk (u16/fp16, L = mw*TP):
  1. DMA-broadcast fm u16 rows across each token's 32 t2-partitions.
  2. DVE TT   xor   u16 (2x mode)            X = fm ^ base
  3. DVE TS   add 1, fp16 writeback (4x)     V = fp16(X+1)   [value conv]
  4. DVE TS   bits>>10, fp16 writeback (4x)  E = fp16 value of (e+15)
  5. ACT per tp slice: out = E*scale + bias  (per-partition scale/bias APs)
  6. DMA out fp16.
The ori column (fm=0) is filled on the host; fp16 RNE boundary cases
(X+1 in the half-ulp window below a power of two, ~0.05%) and the
negated-candidate==0 sign exceptions are patched exactly on the host.
"""

from contextlib import ExitStack

import numpy as np

import concourse.bacc as bacc
import concourse.mybir as mybir
import concourse.tile as tile

H = 16
TP = 8
K = 64
M = 2 * H * K + 1  # 2049
B, T1, T2 = 4, 16, 32
NTOK = B * T1      # 64
NCORE = 8
TPC = NTOK // NCORE   # tokens per core = 8
TOK_ST = 4            # tokens per supertile (4*32 = 128 partitions)
NST = TPC // TOK_ST   # supertiles per core = 2
MD = M - 1            # device m-columns (ori column done on host) = 2048
FWD = MD * TP         # free width = 16384

F32 = mybir.dt.float32
F16 = mybir.dt.float16
U16 = mybir.dt.uint16


def _exp_log2p1(x):
    """floor(log2(x+1)) for integer array x >= 0, exact via f64 frexp."""
    return (np.frexp((np.asarray(x, np.int64) + 1).astype(np.float64))[1] - 1).astype(
        np.int32
    )


def _ddiff_lut():
    """e_dev - e_true per X in [0,65536): device computes e via fp16(X+1)."""
    xs = np.arange(65536, dtype=np.int64)
    e_true = _exp_log2p1(xs)
    bits = np.float16((xs + 1).astype(np.float32)).view(np.uint16)
    e_dev = (bits >> 10).astype(np.int32) - 15
    return (e_dev - e_true).astype(np.int8), e_true.astype(np.int8)


def _host_prep(sta_loc, pos_loc, val_n, rand_raw, perm):
    f32 = np.float32
    sta = np.asarray(sta_loc).reshape(NTOK, TP)
    pos = np.asarray(pos_loc)                      # [B,T2,TP]
    valn = np.asarray(val_n, np.float32).reshape(NTOK, T2)
    perm = np.asarray(perm).astype(np.int64)

    ori = np.abs(sta).astype(np.int64)             # [NTOK,TP]
    posmag = np.abs(pos).astype(np.int64)          # [B,T2,TP]
    psign = np.where(pos >= 0, f32(1.0), f32(-1.0))

    # fm candidate xor-deltas per structural index: [NTOK, H*K, TP]
    hbits = np.arange(H, dtype=np.int64)
    fm_pre = ((np.int64(1) << hbits)[None, :, None, None]
              | (np.asarray(rand_raw) & ((np.int64(1) << hbits) - 1)[None, :, None, None]
                 )).reshape(NTOK, H * K, TP)

    # device column j covers perm position pos_list[j] (ascending, ori slot
    # jo excluded); orig structural index oi = perm[pos_list], fm index
    # oi (oi<H*K: + candidates) or oi-(H*K+1) (- candidates).
    jo = int(np.nonzero(perm == H * K)[0][0])
    pos_list = np.concatenate([np.arange(jo), np.arange(jo + 1, M)])
    oi = perm[pos_list]
    idx2 = np.where(oi < H * K, oi, oi - (H * K + 1))
    sgn_dev = np.where(oi < H * K, f32(1.0), f32(-1.0))       # [MD]
    fm_dev = fm_pre[:, idx2, :].astype(np.uint16)             # [NTOK,MD,TP]

    # host distances sta<->pos (tiny), mirroring reference f32 order
    pm_tok = posmag[np.arange(NTOK) // T1]         # [NTOK,T2,TP]
    ps_tok = psign[np.arange(NTOK) // T1]          # [NTOK,T2,TP]
    ssign = np.where(sta >= 0, f32(1.0), f32(-1.0))
    e_sp = _exp_log2p1(ori[:, None, :] ^ pm_tok)
    s_sp = ((e_sp + 1).astype(np.float32) / f32(H))
    dis_sta = (ssign[:, None, :] * ps_tok) * (f32(1.0) - s_sp) * valn[:, :, None]
    dis_sum = dis_sta.sum(axis=-1, dtype=np.float32)
    A = dis_sum[:, :, None] - dis_sta              # [NTOK,T2,TP] f32
    bias_raw = A * f32(1.0 / TP)
    # out = scale_run*(e+15) + (bias_raw - 30*scale_run),
    # scale_run = -sgn*ps*valn/(H*TP)
    scale_p = -(ps_tok * valn[:, :, None]) * f32(1.0 / (H * TP))  # sgn=+1
    base = (ori[:, None, :] ^ pm_tok).astype(np.uint16)           # [NTOK,T2,TP]

    # sign exceptions: structurally-negated candidate whose value is 0
    exc = np.argwhere((sgn_dev[None, :, None] < 0)
                      & (fm_dev == ori[:, None, :].astype(np.uint16)))

    return dict(fm_dev=fm_dev, sgn_m=sgn_dev, exc=exc, base=base, jo=jo,
                bias=bias_raw, scale_p=scale_p, pm_tok=pm_tok, ps_tok=ps_tok,
                valn=valn, A=A)


def _runs_of_sign(sgn):
    """[(start, end, sign), ...] contiguous runs of sgn (length MD)."""
    runs = []
    s = 0
    n = len(sgn)
    for i in range(1, n + 1):
        if i == n or sgn[i] != sgn[s]:
            runs.append((s, i, float(sgn[s])))
            s = i
    return runs


def _chunks_for(sgn_m, max_w=1024):
    """(c0, c1, sign) chunks over device columns.  Path A: few sign runs ->
    single-sign chunks, sign folded into host ACT scale/bias.  Path B:
    sign=None chunks, device multiplies by a sgn row (extra DVE pass)."""
    runs = _runs_of_sign(sgn_m)
    if len(runs) <= 8:
        chunks = []
        for s, e, g in runs:
            for c0 in range(s, e, max_w):
                chunks.append((c0, min(c0 + max_w, e), g))
        return chunks, True
    chunks = [(c0, min(c0 + max_w, MD), None) for c0 in range(0, MD, max_w)]
    return chunks, False


def _build_program(chunks, path_a, reps=1):
    nc = bacc.Bacc("TRN2", target_bir_lowering=False, debug=False)

    fm_h = nc.dram_tensor("fm", [TPC, FWD], U16, kind="ExternalInput")
    base_h = nc.dram_tensor("base", [NST, 128, TP], U16, kind="ExternalInput")
    scalep_h = nc.dram_tensor("scalep", [NST, 128, TP], F32, kind="ExternalInput")
    biasp_h = nc.dram_tensor("biasp", [NST, 128, TP], F32, kind="ExternalInput")
    scalen_h = (nc.dram_tensor("scalen", [NST, 128, TP], F32, kind="ExternalInput")
                if path_a else None)
    biasn_h = (nc.dram_tensor("biasn", [NST, 128, TP], F32, kind="ExternalInput")
               if path_a else None)
    sgn_h = None if path_a else nc.dram_tensor("sgn", [MD], F16, kind="ExternalInput")
    out_h = nc.dram_tensor("out", [NST, 128, FWD], F16, kind="ExternalOutput")

    with tile.TileContext(nc) as tc, ExitStack() as ctx:
        cpool = ctx.enter_context(tc.tile_pool(name="consts", bufs=1))
        fmpool = ctx.enter_context(tc.tile_pool(name="fm", bufs=3))
        opool = ctx.enter_context(tc.tile_pool(name="outs", bufs=3))

        base_t = cpool.tile([128, NST * TP], U16)
        scalep_t = cpool.tile([128, NST * TP], F32)
        biasp_t = cpool.tile([128, NST * TP], F32)
        for st in range(NST):
            nc.sync.dma_start(base_t[:, st * TP:(st + 1) * TP], base_h.ap()[st])
            nc.sync.dma_start(scalep_t[:, st * TP:(st + 1) * TP], scalep_h.ap()[st])
            nc.sync.dma_start(biasp_t[:, st * TP:(st + 1) * TP], biasp_h.ap()[st])
        if path_a:
            scalen_t = cpool.tile([128, NST * TP], F32)
            biasn_t = cpool.tile([128, NST * TP], F32)
            for st in range(NST):
                nc.sync.dma_start(scalen_t[:, st * TP:(st + 1) * TP],
                                  scalen_h.ap()[st])
                nc.sync.dma_start(biasn_t[:, st * TP:(st + 1) * TP],
                                  biasn_h.ap()[st])
        else:
            sgn_t = cpool.tile([128, MD], F16)
            nc.sync.dma_start(
                sgn_t[:], sgn_h.ap().unsqueeze(0).to_broadcast((128, MD)))

        def one_chunk(st, c0, c1, g):
            mw = c1 - c0
            L = mw * TP
            fm_t = fmpool.tile([128, L], U16, tag="fm")
            src = (fm_h.ap()[st * TOK_ST:(st + 1) * TOK_ST, c0 * TP:c1 * TP]
                   .unsqueeze(1).to_broadcast((TOK_ST, T2, L)))
            nc.sync.dma_start(fm_t[:], src)

            fm3 = fm_t[:].rearrange("p (m t) -> p m t", t=TP)
            base_b = (base_t[:, st * TP:(st + 1) * TP]
                      .unsqueeze(1).to_broadcast((128, mw, TP)))
            nc.vector.tensor_tensor(
                fm3, fm3, base_b, mybir.AluOpType.bitwise_xor)

            v16 = fm_t[:].bitcast(F16)
            # fp16(X+1): u16 ALU input, +1, fp16 value writeback
            nc.vector.tensor_scalar(
                v16, fm_t[:], 1, None, mybir.AluOpType.add)
            # e+15 = fp16 bits >> 10 (bitVec op: u16 -> u16, no cast allowed)
            nc.vector.tensor_scalar(
                fm_t[:], fm_t[:], 10, None, mybir.AluOpType.logical_shift_right)
            # int -> fp16 value convert (ACT input must be float)
            nc.vector.tensor_copy(v16, fm_t[:])
            s3 = v16.rearrange("p (m t) -> p m t", t=TP)
            if not path_a:
                # x = (E - 30) * sgn = sgn*(e-15); uniform scale/bias after
                sgn_b = (sgn_t[:, c0:c1].unsqueeze(2)
                         .to_broadcast((128, mw, TP)))
                nc.vector.scalar_tensor_tensor(
                    s3, s3, 30.0, sgn_b,
                    mybir.AluOpType.subtract, mybir.AluOpType.mult)

            out_t = opool.tile([128, L], F16, tag="out")
            o3 = out_t[:].rearrange("p (m t) -> p m t", t=TP)
            sc_t = scalep_t if (g is None or g > 0) else scalen_t
            bi_t = biasp_t if (g is None or g > 0) else biasn_t
            for tp in range(TP):
                j = st * TP + tp
                nc.scalar.activation(
                    o3[:, :, tp], s3[:, :, tp],
                    mybir.ActivationFunctionType.Identity,
                    bias=bi_t[:, j:j + 1], scale=sc_t[:, j:j + 1])
            nc.sync.dma_start(out_h.ap()[st, :, c0 * TP:c1 * TP], out_t[:])

        for _rep in range(reps):
            for st in range(NST):
                for (c0, c1, g) in chunks:
                    one_chunk(st, c0, c1, g)

    nc.compile()
    return nc


def _in_maps(prep, path_a):
    """Per-core input dicts."""
    fm_dev, base = prep["fm_dev"], prep["base"]
    bias, scale_p = prep["bias"], prep["scale_p"]
    maps = []
    for c in range(NCORE):
        t0 = c * TPC
        d = {
            "fm": fm_dev[t0:t0 + TPC].reshape(TPC, FWD),
            "base": base[t0:t0 + TPC].reshape(NST, 128, TP),
            "scalep": scale_p[t0:t0 + TPC].reshape(NST, 128, TP),
        }
        if path_a:
            d["scalen"] = -d["scalep"]
            d["biasp"] = (bias[t0:t0 + TPC].reshape(NST, 128, TP)
                          - np.float32(30.0) * d["scalep"])
            d["biasn"] = (bias[t0:t0 + TPC].reshape(NST, 128, TP)
                          - np.float32(30.0) * d["scalen"])
        else:
            # device computes x = sgn*(e-15); out = scale_p*x + bias
            d["biasp"] = bias[t0:t0 + TPC].reshape(NST, 128, TP)
            d["sgn"] = prep["sgn_m"].astype(np.float16)
        maps.append(d)
    return maps


def _patch_and_assemble(dev_f32, prep):
    """dev_f32: [NTOK, T2, MD, TP] f32 device result (fp16-upcast).
    Patch fp16-rounding boundary cases + sign exceptions exactly, insert
    the ori column, scatter device columns to perm positions."""
    f32 = np.float32
    ddiff, e_true = _ddiff_lut()
    X = prep["base"][:, :, None, :] ^ prep["fm_dev"][:, None, :, :]
    bad = np.nonzero(ddiff[X])
    if bad[0].size:
        xt, t2, jc, tp = bad
        et = e_true[X[bad]].astype(np.float32)
        s = (et + f32(1.0)) / f32(H)
        dis = (prep["sgn_m"][jc] * prep["ps_tok"][xt, t2, tp]
               * (f32(1.0) - s) * prep["valn"][xt, t2])
        dev_f32[bad] = (prep["A"][xt, t2, tp] + dis) * f32(1.0 / TP)

    # negated candidate that is actually 0: sign is +1, X = posmag
    for tok, jc, tp in prep["exc"]:
        pm = prep["pm_tok"][tok, :, tp]            # [T2]
        ps = prep["ps_tok"][tok, :, tp]
        s0 = (_exp_log2p1(pm) + 1).astype(np.float32) / f32(H)
        dis_cnc = ps * (f32(1.0) - s0) * prep["valn"][tok]
        dev_f32[tok, :, jc, tp] = (prep["A"][tok, :, tp] + dis_cnc) * f32(1.0 / TP)

    out = np.empty((NTOK, T2, M, TP), np.float32)
    jo = prep["jo"]
    out[:, :, :jo, :] = dev_f32[:, :, :jo, :]
    out[:, :, jo + 1:, :] = dev_f32[:, :, jo:, :]
    # ori column: candidate = ori (sign +1), X = base
    e0 = _exp_log2p1(prep["base"].astype(np.int64))
    s0 = (e0 + 1).astype(np.float32) / f32(H)
    dis0 = prep["ps_tok"] * (f32(1.0) - s0) * prep["valn"][:, :, None]
    out[:, :, jo, :] = (prep["A"] + dis0) * f32(1.0 / TP)
    return out


def kernel(sta_loc, pos_loc, val_n, rand_raw, perm, _sim=False):
    prep = _host_prep(sta_loc, pos_loc, val_n, rand_raw, perm)
    chunks, path_a = _chunks_for(prep["sgn_m"])
    nc = _build_program(chunks, path_a)
    maps = _in_maps(prep, path_a)

    if _sim:
        from concourse.bass_interp import CoreSim
        results = []
        for c in range(NCORE):
            sim = CoreSim(nc, trace=False)
            for k, v in maps[c].items():
                sim.tensor(k)[:] = v
            sim.simulate(check_with_hw=False)
            results.append({"out": np.array(sim.tensor("out"))})
    else:
        from concourse.bass_utils import run_bass_kernel_spmd
        res = run_bass_kernel_spmd(nc, maps, list(range(NCORE)))
        results = res.results

    dev = np.empty((NTOK, T2, MD, TP), np.float32)
    for c in range(NCORE):
        o = np.asarray(results[c]["out"]).reshape(NST, TOK_ST, T2, FWD)
        for st in range(NST):
            tok0 = c * TPC + st * TOK_ST
            dev[tok0:tok0 + TOK_ST] = o[st].reshape(
                TOK_ST, T2, MD, TP).astype(np.float32)
    out = _patch_and_assemble(dev, prep)
    return out.reshape(B, T1, T2, M, TP)


if __name__ == "__main__":
    pass


# revision 4
# speedup vs baseline: 11.7378x; 11.7378x over previous
"""Trainium2 Bass kernel for nn_CritiGraph (ct_val expansion), 16-bit pipeline.

Math: ct_val[b,t1,t2,m,tp] = (dis_sum - dis_sta_pos + dis_cnc_pos)/TP with
dis(c1,c2,norm) = sign(c1)sign(c2) * (1 - table[|c1|^|c2|]) * norm and
table[x] = (floor(log2(x+1))+1)/16.  With X = |cnc|^|pos| = base ^ fm
(base[t2,tp] = ori^|pos|, fm[m,tp] the candidate xor-delta) and
e = floor(log2(X+1)):  out = scale_run*(e+15) + bias_run, where
scale_run = -sgn*ps*valn/(H*TP) and bias_run = A/TP - 30*scale_run are
per-(tok,t2,tp) scalars (sgn structural per m-run).

Device pipeline per [128, L] chunk (u16/fp16, L = mw*TP):
  1. DMA-broadcast fm u16 rows across each token's 32 t2-partitions.
  2. DVE TT  xor as packed u32 pairs (in1 = per-partition base pair,
     free-broadcast) -> X
  3. DVE TS  add 1, fp16 writeback: V = fp16(X+1)  [int->value conv]
  4. DVE TS  packed u32 (bits>>10) & 0x1F001F: u16 halves = e+15
  5. ACT per tp slice on the u16 input: out = (e+15)*scale + bias, fp16
  6. DMA out fp16.
The ori column (fm=0) is filled on the host; fp16 RNE boundary cases
(X+1 in the half-ulp window below a power of two, ~0.05%) and the
negated-candidate==0 sign exceptions are patched exactly on the host.
"""

from contextlib import ExitStack

import numpy as np

import concourse.bacc as bacc
import concourse.mybir as mybir
import concourse.tile as tile

H = 16
TP = 8
K = 64
M = 2 * H * K + 1  # 2049
B, T1, T2 = 4, 16, 32
NTOK = B * T1      # 64
NCORE = 8
TPC = NTOK // NCORE   # tokens per core = 8
TOK_ST = 4            # tokens per supertile (4*32 = 128 partitions)
NST = TPC // TOK_ST   # supertiles per core = 2
MD = M - 1            # device m-columns (ori column done on host) = 2048
FWD = MD * TP         # free width = 16384

TP_MAJOR = False      # free dim order: False -> (m, tp); True -> (tp, m)

F32 = mybir.dt.float32
F16 = mybir.dt.float16
U16 = mybir.dt.uint16
U32 = mybir.dt.uint32


def _exp_log2p1(x):
    """floor(log2(x+1)) for integer array x >= 0, exact via f64 frexp."""
    return (np.frexp((np.asarray(x, np.int64) + 1).astype(np.float64))[1] - 1).astype(
        np.int32
    )


def _ddiff_lut():
    """e_dev - e_true per X in [0,65536): device computes e via fp16(X+1)."""
    xs = np.arange(65536, dtype=np.int64)
    e_true = _exp_log2p1(xs)
    with np.errstate(over="ignore"):
        bits = np.float16((xs + 1).astype(np.float32)).view(np.uint16)
    e_dev = (bits >> 10).astype(np.int32) - 15
    return (e_dev - e_true).astype(np.int8), e_true.astype(np.int8)


def _host_prep(sta_loc, pos_loc, val_n, rand_raw, perm):
    f32 = np.float32
    sta = np.asarray(sta_loc).reshape(NTOK, TP)
    pos = np.asarray(pos_loc)                      # [B,T2,TP]
    valn = np.asarray(val_n, np.float32).reshape(NTOK, T2)
    perm = np.asarray(perm).astype(np.int64)

    ori = np.abs(sta).astype(np.int64)             # [NTOK,TP]
    posmag = np.abs(pos).astype(np.int64)          # [B,T2,TP]
    psign = np.where(pos >= 0, f32(1.0), f32(-1.0))

    # fm candidate xor-deltas per structural index: [NTOK, H*K, TP]
    hbits = np.arange(H, dtype=np.int64)
    fm_pre = ((np.int64(1) << hbits)[None, :, None, None]
              | (np.asarray(rand_raw) & ((np.int64(1) << hbits) - 1)[None, :, None, None]
                 )).reshape(NTOK, H * K, TP)

    # device column j covers perm position pos_list[j] (ascending, ori slot
    # jo excluded); orig structural index oi = perm[pos_list].
    jo = int(np.nonzero(perm == H * K)[0][0])
    pos_list = np.concatenate([np.arange(jo), np.arange(jo + 1, M)])
    oi = perm[pos_list]
    idx2 = np.where(oi < H * K, oi, oi - (H * K + 1))
    sgn_dev = np.where(oi < H * K, f32(1.0), f32(-1.0))       # [MD]
    fm_dev = fm_pre[:, idx2, :].astype(np.uint16)             # [NTOK,MD,TP]

    # host distances sta<->pos (tiny), mirroring reference f32 order
    pm_tok = posmag[np.arange(NTOK) // T1]         # [NTOK,T2,TP]
    ps_tok = psign[np.arange(NTOK) // T1]          # [NTOK,T2,TP]
    ssign = np.where(sta >= 0, f32(1.0), f32(-1.0))
    e_sp = _exp_log2p1(ori[:, None, :] ^ pm_tok)
    s_sp = ((e_sp + 1).astype(np.float32) / f32(H))
    dis_sta = (ssign[:, None, :] * ps_tok) * (f32(1.0) - s_sp) * valn[:, :, None]
    dis_sum = dis_sta.sum(axis=-1, dtype=np.float32)
    A = dis_sum[:, :, None] - dis_sta              # [NTOK,T2,TP] f32
    bias_raw = A * f32(1.0 / TP)
    # out = scale_run*(e+15) + (bias_raw - 30*scale_run),
    # scale_run = -sgn*ps*valn/(H*TP)
    scale_p = -(ps_tok * valn[:, :, None]) * f32(1.0 / (H * TP))  # sgn=+1
    base = (ori[:, None, :] ^ pm_tok).astype(np.uint16)           # [NTOK,T2,TP]
    b32 = base.astype(np.uint32)
    if TP_MAJOR:
        base_pair = (b32 << 16) | b32                             # [NTOK,T2,TP]
    else:
        base_pair = (b32[..., 1::2] << 16) | b32[..., 0::2]       # [NTOK,T2,TP/2]

    # sign exceptions: structurally-negated candidate whose value is 0
    exc = np.argwhere((sgn_dev[None, :, None] < 0)
                      & (fm_dev == ori[:, None, :].astype(np.uint16)))

    return dict(fm_dev=fm_dev, sgn_m=sgn_dev, exc=exc, base=base, jo=jo,
                base_pair=base_pair, bias=bias_raw, scale_p=scale_p,
                pm_tok=pm_tok, ps_tok=ps_tok, valn=valn, A=A)


def _runs_of_sign(sgn):
    """[(start, end, sign), ...] contiguous runs of sgn (length MD)."""
    runs = []
    s = 0
    n = len(sgn)
    for i in range(1, n + 1):
        if i == n or sgn[i] != sgn[s]:
            runs.append((s, i, float(sgn[s])))
            s = i
    return runs


def _chunks_for(sgn_m, max_w=1024):
    """(c0, c1, sign) chunks over device columns.  Path A: few sign runs ->
    single-sign chunks, sign folded into host ACT scale/bias.  Path B:
    sign=None chunks, device multiplies by a sgn row (extra DVE pass).
    Chunk bounds must be even (packed-u32 xor pairs along m in TP_MAJOR)."""
    runs = _runs_of_sign(sgn_m)
    ok_parity = all((s % 2 == 0 and e % 2 == 0) for s, e, _ in runs[:-1]) \
        if TP_MAJOR else True
    if len(runs) <= 8 and ok_parity:
        chunks = []
        for s, e, g in runs:
            for c0 in range(s, e, max_w):
                chunks.append((c0, min(c0 + max_w, e), g))
        return chunks, True
    chunks = [(c0, min(c0 + max_w, MD), None) for c0 in range(0, MD, max_w)]
    return chunks, False


NPAIR = TP if TP_MAJOR else TP // 2


def _build_program(chunks, path_a, reps=1):
    nc = bacc.Bacc("TRN2", target_bir_lowering=False, debug=False)

    fm_h = nc.dram_tensor("fm", [TPC, FWD], U16, kind="ExternalInput")
    bp_h = nc.dram_tensor("basep", [NST, 128, NPAIR], U32, kind="ExternalInput")
    scalep_h = nc.dram_tensor("scalep", [NST, 128, TP], F32, kind="ExternalInput")
    biasp_h = nc.dram_tensor("biasp", [NST, 128, TP], F32, kind="ExternalInput")
    scalen_h = (nc.dram_tensor("scalen", [NST, 128, TP], F32, kind="ExternalInput")
                if path_a else None)
    biasn_h = (nc.dram_tensor("biasn", [NST, 128, TP], F32, kind="ExternalInput")
               if path_a else None)
    sgn_h = None if path_a else nc.dram_tensor("sgn", [MD], F16, kind="ExternalInput")
    out_h = nc.dram_tensor("out", [NST, 128, FWD], F16, kind="ExternalOutput")

    with tile.TileContext(nc) as tc, ExitStack() as ctx:
        cpool = ctx.enter_context(tc.tile_pool(name="consts", bufs=1))
        fmpool = ctx.enter_context(tc.tile_pool(name="fm", bufs=3))
        opool = ctx.enter_context(tc.tile_pool(name="outs", bufs=3))

        bp_t = cpool.tile([128, NST * NPAIR], U32)
        scalep_t = cpool.tile([128, NST * TP], F32)
        biasp_t = cpool.tile([128, NST * TP], F32)
        for st in range(NST):
            nc.sync.dma_start(bp_t[:, st * NPAIR:(st + 1) * NPAIR], bp_h.ap()[st])
            nc.sync.dma_start(scalep_t[:, st * TP:(st + 1) * TP], scalep_h.ap()[st])
            nc.sync.dma_start(biasp_t[:, st * TP:(st + 1) * TP], biasp_h.ap()[st])
        if path_a:
            scalen_t = cpool.tile([128, NST * TP], F32)
            biasn_t = cpool.tile([128, NST * TP], F32)
            for st in range(NST):
                nc.sync.dma_start(scalen_t[:, st * TP:(st + 1) * TP],
                                  scalen_h.ap()[st])
                nc.sync.dma_start(biasn_t[:, st * TP:(st + 1) * TP],
                                  biasn_h.ap()[st])
        else:
            sgn_t = cpool.tile([128, MD], F16)
            nc.sync.dma_start(
                sgn_t[:], sgn_h.ap().unsqueeze(0).to_broadcast((128, MD)))

        def one_chunk(st, c0, c1, g):
            mw = c1 - c0
            L = mw * TP
            fm_t = fmpool.tile([128, L], U16, tag="fm")
            src = (fm_h.ap()[st * TOK_ST:(st + 1) * TOK_ST, c0 * TP:c1 * TP]
                   .unsqueeze(1).to_broadcast((TOK_ST, T2, L)))
            nc.sync.dma_start(fm_t[:], src)

            # xor as packed u32 pairs; base pair broadcast along m
            fmp = fm_t[:].bitcast(U32)
            if TP_MAJOR:
                fmp3 = fmp.rearrange("p (t m) -> p t m", t=TP)
                bp_b = (bp_t[:, st * NPAIR:(st + 1) * NPAIR]
                        .unsqueeze(2).to_broadcast((128, TP, mw // 2)))
            else:
                fmp3 = fmp.rearrange("p (m t) -> p m t", t=TP // 2)
                bp_b = (bp_t[:, st * NPAIR:(st + 1) * NPAIR]
                        .unsqueeze(1).to_broadcast((128, mw, TP // 2)))
            nc.vector.tensor_tensor(
                fmp3, fmp3, bp_b, mybir.AluOpType.bitwise_xor)

            v16 = fm_t[:].bitcast(F16)
            # fp16(X+1): u16 ALU input, +1, fp16 value writeback
            nc.vector.tensor_scalar(
                v16, fm_t[:], 1, None, mybir.AluOpType.add)
            # e+15 in each u16 half: packed (bits >> 10) & 0x1F001F
            nc.vector.tensor_scalar(
                fmp, fmp, 10, 0x001F001F,
                mybir.AluOpType.logical_shift_right,
                mybir.AluOpType.bitwise_and)
            if TP_MAJOR:
                s3 = fm_t[:].rearrange("p (t m) -> p t m", t=TP)
            else:
                s3 = fm_t[:].rearrange("p (m t) -> p m t", t=TP)
            if not path_a:
                # x = (E - 30) * sgn = sgn*(e-15): needs float input; use
                # an extra convert into the out tile below instead.
                pass

            out_t = opool.tile([128, L], F16, tag="out")
            sc_t = scalep_t if (g is None or g > 0) else scalen_t
            bi_t = biasp_t if (g is None or g > 0) else biasn_t
            if path_a:
                if TP_MAJOR:
                    o3 = out_t[:].rearrange("p (t m) -> p t m", t=TP)
                    for tp in range(TP):
                        j = st * TP + tp
                        nc.scalar.activation(
                            o3[:, tp, :], s3[:, tp, :],
                            mybir.ActivationFunctionType.Identity,
                            bias=bi_t[:, j:j + 1], scale=sc_t[:, j:j + 1])
                else:
                    o3 = out_t[:].rearrange("p (m t) -> p m t", t=TP)
                    for tp in range(TP):
                        j = st * TP + tp
                        nc.scalar.activation(
                            o3[:, :, tp], s3[:, :, tp],
                            mybir.ActivationFunctionType.Identity,
                            bias=bi_t[:, j:j + 1], scale=sc_t[:, j:j + 1])
            else:
                # path B: convert u16 -> fp16 value, fold sign, then ACT
                ov = out_t[:].bitcast(F16)
                nc.vector.tensor_copy(ov, fm_t[:])
                s3f = ov.rearrange("p (m t) -> p m t", t=TP)
                sgn_b = (sgn_t[:, c0:c1].unsqueeze(2)
                         .to_broadcast((128, mw, TP)))
                nc.vector.scalar_tensor_tensor(
                    s3f, s3f, 30.0, sgn_b,
                    mybir.AluOpType.subtract, mybir.AluOpType.mult)
                o3 = out_t[:].rearrange("p (m t) -> p m t", t=TP)
                for tp in range(TP):
                    j = st * TP + tp
                    nc.scalar.activation(
                        o3[:, :, tp], o3[:, :, tp],
                        mybir.ActivationFunctionType.Identity,
                        bias=bi_t[:, j:j + 1], scale=sc_t[:, j:j + 1])
            nc.sync.dma_start(out_h.ap()[st, :, c0 * TP:c1 * TP], out_t[:])

        for _rep in range(reps):
            for st in range(NST):
                for (c0, c1, g) in chunks:
                    one_chunk(st, c0, c1, g)

    nc.compile()
    return nc


def _in_maps(prep, path_a):
    """Per-core input dicts."""
    fm_dev, base_pair = prep["fm_dev"], prep["base_pair"]
    bias, scale_p = prep["bias"], prep["scale_p"]
    maps = []
    for c in range(NCORE):
        t0 = c * TPC
        fm = fm_dev[t0:t0 + TPC]                    # [TPC, MD, TP]
        if TP_MAJOR:
            fm = fm.transpose(0, 2, 1)              # [TPC, TP, MD]
        d = {
            "fm": np.ascontiguousarray(fm).reshape(TPC, FWD),
            "basep": base_pair[t0:t0 + TPC].reshape(NST, 128, NPAIR),
            "scalep": scale_p[t0:t0 + TPC].reshape(NST, 128, TP),
        }
        if path_a:
            d["scalen"] = -d["scalep"]
            d["biasp"] = (bias[t0:t0 + TPC].reshape(NST, 128, TP)
                          - np.float32(30.0) * d["scalep"])
            d["biasn"] = (bias[t0:t0 + TPC].reshape(NST, 128, TP)
                          - np.float32(30.0) * d["scalen"])
        else:
            # device computes x = sgn*(e-15); out = scale_p*x + bias
            d["biasp"] = bias[t0:t0 + TPC].reshape(NST, 128, TP)
            d["sgn"] = prep["sgn_m"].astype(np.float16)
        maps.append(d)
    return maps


def _patch_and_assemble(dev_f32, prep):
    """dev_f32: [NTOK, T2, MD, TP] f32 device result (fp16-upcast).
    Patch fp16-rounding boundary cases + sign exceptions exactly, insert
    the ori column, scatter device columns to perm positions."""
    f32 = np.float32
    ddiff, e_true = _ddiff_lut()
    X = prep["base"][:, :, None, :] ^ prep["fm_dev"][:, None, :, :]
    bad = np.nonzero(ddiff[X])
    if bad[0].size:
        xt, t2, jc, tp = bad
        et = e_true[X[bad]].astype(np.float32)
        s = (et + f32(1.0)) / f32(H)
        dis = (prep["sgn_m"][jc] * prep["ps_tok"][xt, t2, tp]
               * (f32(1.0) - s) * prep["valn"][xt, t2])
        dev_f32[bad] = (prep["A"][xt, t2, tp] + dis) * f32(1.0 / TP)

    # negated candidate that is actually 0: sign is +1, X = posmag
    for tok, jc, tp in prep["exc"]:
        pm = prep["pm_tok"][tok, :, tp]            # [T2]
        ps = prep["ps_tok"][tok, :, tp]
        s0 = (_exp_log2p1(pm) + 1).astype(np.float32) / f32(H)
        dis_cnc = ps * (f32(1.0) - s0) * prep["valn"][tok]
        dev_f32[tok, :, jc, tp] = (prep["A"][tok, :, tp] + dis_cnc) * f32(1.0 / TP)

    out = np.empty((NTOK, T2, M, TP), np.float32)
    jo = prep["jo"]
    out[:, :, :jo, :] = dev_f32[:, :, :jo, :]
    out[:, :, jo + 1:, :] = dev_f32[:, :, jo:, :]
    # ori column: candidate = ori (sign +1), X = base
    e0 = _exp_log2p1(prep["base"].astype(np.int64))
    s0 = (e0 + 1).astype(np.float32) / f32(H)
    dis0 = prep["ps_tok"] * (f32(1.0) - s0) * prep["valn"][:, :, None]
    out[:, :, jo, :] = (prep["A"] + dis0) * f32(1.0 / TP)
    return out


def kernel(sta_loc, pos_loc, val_n, rand_raw, perm, _sim=False):
    prep = _host_prep(sta_loc, pos_loc, val_n, rand_raw, perm)
    chunks, path_a = _chunks_for(prep["sgn_m"])
    nc = _build_program(chunks, path_a)
    maps = _in_maps(prep, path_a)

    if _sim:
        from concourse.bass_interp import CoreSim
        results = []
        for c in range(NCORE):
            sim = CoreSim(nc, trace=False)
            for k, v in maps[c].items():
                sim.tensor(k)[:] = v
            sim.simulate(check_with_hw=False)
            results.append({"out": np.array(sim.tensor("out"))})
    else:
        from concourse.bass_utils import run_bass_kernel_spmd
        res = run_bass_kernel_spmd(nc, maps, list(range(NCORE)))
        results = res.results

    dev = np.empty((NTOK, T2, MD, TP), np.float32)
    for c in range(NCORE):
        if TP_MAJOR:
            o = np.asarray(results[c]["out"]).reshape(NST, TOK_ST, T2, TP, MD)
            o = o.transpose(0, 1, 2, 4, 3)
        else:
            o = np.asarray(results[c]["out"]).reshape(NST, TOK_ST, T2, MD, TP)
        for st in range(NST):
            tok0 = c * TPC + st * TOK_ST
            dev[tok0:tok0 + TOK_ST] = o[st].astype(np.float32)
    out = _patch_and_assemble(dev, prep)
    return out.reshape(B, T1, T2, M, TP)


if __name__ == "__main__":
    pass


# revision 6
# speedup vs baseline: 15.3747x; 1.3098x over previous
"""Trainium2 Bass kernel for nn_CritiGraph (ct_val expansion), 16-bit pipeline.

Math: ct_val[b,t1,t2,m,tp] = (dis_sum - dis_sta_pos + dis_cnc_pos)/TP with
dis(c1,c2,norm) = sign(c1)sign(c2) * (1 - table[|c1|^|c2|]) * norm and
table[x] = (floor(log2(x+1))+1)/16.  With X = |cnc|^|pos| = base ^ fm
(base[t2,tp] = ori^|pos|, fm[m,tp] the candidate xor-delta) and
e = floor(log2(X+1)):  out = scale_run*(e+15) + bias_run, where
scale_run = -sgn*ps*valn/(H*TP) and bias_run = A/TP - 30*scale_run are
per-(tok,t2,tp) scalars (sgn structural per m-run).

Device pipeline per [128, L] chunk (u16/fp16, L = mw*TP):
  1. DMA-broadcast fm u16 rows across each token's 32 t2-partitions.
  2. DVE TT  xor as packed u32 pairs (in1 = per-partition base pair,
     free-broadcast) -> X
  3. DVE TS  add 1, fp16 writeback: V = fp16(X+1)  [int->value conv]
  4. DVE TS  packed u32 (bits>>10) & 0x1F001F: u16 halves = e+15
  5. ACT per tp slice on the u16 input: out = (e+15)*scale + bias, fp16
  6. DMA out fp16.
The ori column (fm=0) is filled on the host; fp16 RNE boundary cases
(X+1 in the half-ulp window below a power of two, ~0.05%) and the
negated-candidate==0 sign exceptions are patched exactly on the host.
"""

from contextlib import ExitStack

import numpy as np

import concourse.bacc as bacc
import concourse.mybir as mybir
import concourse.tile as tile

H = 16
TP = 8
K = 64
M = 2 * H * K + 1  # 2049
B, T1, T2 = 4, 16, 32
NTOK = B * T1      # 64
NCORE = 8
TPC = NTOK // NCORE   # tokens per core = 8
TOK_ST = 4            # tokens per supertile (4*32 = 128 partitions)
NST = TPC // TOK_ST   # supertiles per core = 2
MD = M - 1            # device m-columns (ori column done on host) = 2048
FWD = MD * TP         # free width = 16384

TP_MAJOR = False      # free dim order: False -> (m, tp); True -> (tp, m)

F32 = mybir.dt.float32
F16 = mybir.dt.float16
U16 = mybir.dt.uint16
U32 = mybir.dt.uint32


def _exp_log2p1(x):
    """floor(log2(x+1)) for integer array x >= 0, exact via f64 frexp."""
    return (np.frexp((np.asarray(x, np.int64) + 1).astype(np.float64))[1] - 1).astype(
        np.int32
    )


def _ddiff_lut():
    """e_dev - e_true per X in [0,65536): device computes e via fp16(X+1)."""
    xs = np.arange(65536, dtype=np.int64)
    e_true = _exp_log2p1(xs)
    with np.errstate(over="ignore"):
        bits = np.float16((xs + 1).astype(np.float32)).view(np.uint16)
    e_dev = (bits >> 10).astype(np.int32) - 15
    return (e_dev - e_true).astype(np.int8), e_true.astype(np.int8)


def _host_prep(sta_loc, pos_loc, val_n, rand_raw, perm):
    f32 = np.float32
    sta = np.asarray(sta_loc).reshape(NTOK, TP)
    pos = np.asarray(pos_loc)                      # [B,T2,TP]
    valn = np.asarray(val_n, np.float32).reshape(NTOK, T2)
    perm = np.asarray(perm).astype(np.int64)

    ori = np.abs(sta).astype(np.int64)             # [NTOK,TP]
    posmag = np.abs(pos).astype(np.int64)          # [B,T2,TP]
    psign = np.where(pos >= 0, f32(1.0), f32(-1.0))

    # fm candidate xor-deltas per structural index: [NTOK, H*K, TP]
    hbits = np.arange(H, dtype=np.int64)
    fm_pre = ((np.int64(1) << hbits)[None, :, None, None]
              | (np.asarray(rand_raw) & ((np.int64(1) << hbits) - 1)[None, :, None, None]
                 )).reshape(NTOK, H * K, TP)

    # device column j covers perm position pos_list[j] (ascending, ori slot
    # jo excluded); orig structural index oi = perm[pos_list].
    jo = int(np.nonzero(perm == H * K)[0][0])
    pos_list = np.concatenate([np.arange(jo), np.arange(jo + 1, M)])
    oi = perm[pos_list]
    idx2 = np.where(oi < H * K, oi, oi - (H * K + 1))
    sgn_dev = np.where(oi < H * K, f32(1.0), f32(-1.0))       # [MD]
    fm_dev = fm_pre[:, idx2, :].astype(np.uint16)             # [NTOK,MD,TP]

    # host distances sta<->pos (tiny), mirroring reference f32 order
    pm_tok = posmag[np.arange(NTOK) // T1]         # [NTOK,T2,TP]
    ps_tok = psign[np.arange(NTOK) // T1]          # [NTOK,T2,TP]
    ssign = np.where(sta >= 0, f32(1.0), f32(-1.0))
    e_sp = _exp_log2p1(ori[:, None, :] ^ pm_tok)
    s_sp = ((e_sp + 1).astype(np.float32) / f32(H))
    dis_sta = (ssign[:, None, :] * ps_tok) * (f32(1.0) - s_sp) * valn[:, :, None]
    dis_sum = dis_sta.sum(axis=-1, dtype=np.float32)
    A = dis_sum[:, :, None] - dis_sta              # [NTOK,T2,TP] f32
    bias_raw = A * f32(1.0 / TP)
    # out = scale_run*(e+15) + (bias_raw - 30*scale_run),
    # scale_run = -sgn*ps*valn/(H*TP)
    scale_p = -(ps_tok * valn[:, :, None]) * f32(1.0 / (H * TP))  # sgn=+1
    base = (ori[:, None, :] ^ pm_tok).astype(np.uint16)           # [NTOK,T2,TP]
    b32 = base.astype(np.uint32)
    if TP_MAJOR:
        base_pair = (b32 << 16) | b32                             # [NTOK,T2,TP]
    else:
        base_pair = (b32[..., 1::2] << 16) | b32[..., 0::2]       # [NTOK,T2,TP/2]

    # sign exceptions: structurally-negated candidate whose value is 0
    exc = np.argwhere((sgn_dev[None, :, None] < 0)
                      & (fm_dev == ori[:, None, :].astype(np.uint16)))

    return dict(fm_dev=fm_dev, sgn_m=sgn_dev, exc=exc, base=base, jo=jo,
                base_pair=base_pair, bias=bias_raw, scale_p=scale_p,
                pm_tok=pm_tok, ps_tok=ps_tok, valn=valn, A=A)


def _runs_of_sign(sgn):
    """[(start, end, sign), ...] contiguous runs of sgn (length MD)."""
    runs = []
    s = 0
    n = len(sgn)
    for i in range(1, n + 1):
        if i == n or sgn[i] != sgn[s]:
            runs.append((s, i, float(sgn[s])))
            s = i
    return runs


def _chunks_for(sgn_m, max_w=1024):
    """(c0, c1, sign) chunks over device columns.  Path A: few sign runs ->
    single-sign chunks, sign folded into host ACT scale/bias.  Path B:
    sign=None chunks, device multiplies by a sgn row (extra DVE pass).
    Chunk bounds must be even (packed-u32 xor pairs along m in TP_MAJOR)."""
    runs = _runs_of_sign(sgn_m)
    ok_parity = all((s % 2 == 0 and e % 2 == 0) for s, e, _ in runs[:-1]) \
        if TP_MAJOR else True
    if len(runs) <= 8 and ok_parity:
        chunks = []
        for s, e, g in runs:
            for c0 in range(s, e, max_w):
                chunks.append((c0, min(c0 + max_w, e), g))
        return chunks, True
    chunks = [(c0, min(c0 + max_w, MD), None) for c0 in range(0, MD, max_w)]
    return chunks, False


NPAIR = TP if TP_MAJOR else TP // 2


def _build_program(chunks, path_a, reps=1):
    nc = bacc.Bacc("TRN2", target_bir_lowering=False, debug=False)

    fm_h = nc.dram_tensor("fm", [TPC, FWD], U16, kind="ExternalInput")
    bp_h = nc.dram_tensor("basep", [NST, 128, NPAIR], U32, kind="ExternalInput")
    scalep_h = nc.dram_tensor("scalep", [NST, 128, TP], F32, kind="ExternalInput")
    biasp_h = nc.dram_tensor("biasp", [NST, 128, TP], F32, kind="ExternalInput")
    scalen_h = (nc.dram_tensor("scalen", [NST, 128, TP], F32, kind="ExternalInput")
                if path_a else None)
    biasn_h = (nc.dram_tensor("biasn", [NST, 128, TP], F32, kind="ExternalInput")
               if path_a else None)
    sgn_h = None if path_a else nc.dram_tensor("sgn", [MD], F16, kind="ExternalInput")
    out_h = nc.dram_tensor("out", [NST, 128, FWD], F16, kind="ExternalOutput")

    with tile.TileContext(nc) as tc, ExitStack() as ctx:
        cpool = ctx.enter_context(tc.tile_pool(name="consts", bufs=1))
        fmpool = ctx.enter_context(tc.tile_pool(name="fm", bufs=4))
        opool = ctx.enter_context(tc.tile_pool(name="outs", bufs=4))

        bp_t = cpool.tile([128, NST * NPAIR], U32)
        scalep_t = cpool.tile([128, NST * TP], F32)
        biasp_t = cpool.tile([128, NST * TP], F32)
        for st in range(NST):
            nc.sync.dma_start(bp_t[:, st * NPAIR:(st + 1) * NPAIR], bp_h.ap()[st])
            nc.sync.dma_start(scalep_t[:, st * TP:(st + 1) * TP], scalep_h.ap()[st])
            nc.sync.dma_start(biasp_t[:, st * TP:(st + 1) * TP], biasp_h.ap()[st])
        if path_a:
            scalen_t = cpool.tile([128, NST * TP], F32)
            biasn_t = cpool.tile([128, NST * TP], F32)
            for st in range(NST):
                nc.sync.dma_start(scalen_t[:, st * TP:(st + 1) * TP],
                                  scalen_h.ap()[st])
                nc.sync.dma_start(biasn_t[:, st * TP:(st + 1) * TP],
                                  biasn_h.ap()[st])
        else:
            sgn_t = cpool.tile([128, MD], F16)
            nc.sync.dma_start(
                sgn_t[:], sgn_h.ap().unsqueeze(0).to_broadcast((128, MD)))

        def one_chunk(st, c0, c1, g):
            mw = c1 - c0
            L = mw * TP
            fm_t = fmpool.tile([128, L], U16, tag="fm")
            src = (fm_h.ap()[st * TOK_ST:(st + 1) * TOK_ST, c0 * TP:c1 * TP]
                   .unsqueeze(1).to_broadcast((TOK_ST, T2, L)))
            # loads on the ACT HWDGE ring, stores on the SP ring: the two
            # rings drain in parallel (HWDGE is FIFO per issuing engine)
            nc.scalar.dma_start(fm_t[:], src)

            # xor as packed u32 pairs; base pair broadcast along m
            fmp = fm_t[:].bitcast(U32)
            if TP_MAJOR:
                fmp3 = fmp.rearrange("p (t m) -> p t m", t=TP)
                bp_b = (bp_t[:, st * NPAIR:(st + 1) * NPAIR]
                        .unsqueeze(2).to_broadcast((128, TP, mw // 2)))
            else:
                fmp3 = fmp.rearrange("p (m t) -> p m t", t=TP // 2)
                bp_b = (bp_t[:, st * NPAIR:(st + 1) * NPAIR]
                        .unsqueeze(1).to_broadcast((128, mw, TP // 2)))
            nc.vector.tensor_tensor(
                fmp3, fmp3, bp_b, mybir.AluOpType.bitwise_xor)

            v16 = fm_t[:].bitcast(F16)
            # fp16(X+1): u16 ALU input, +1, fp16 value writeback
            nc.vector.tensor_scalar(
                v16, fm_t[:], 1, None, mybir.AluOpType.add)
            # e+15 in each u16 half: packed (bits >> 10) & 0x1F001F
            nc.vector.tensor_scalar(
                fmp, fmp, 10, 0x001F001F,
                mybir.AluOpType.logical_shift_right,
                mybir.AluOpType.bitwise_and)
            if TP_MAJOR:
                s3 = fm_t[:].rearrange("p (t m) -> p t m", t=TP)
            else:
                s3 = fm_t[:].rearrange("p (m t) -> p m t", t=TP)
            if not path_a:
                # x = (E - 30) * sgn = sgn*(e-15): needs float input; use
                # an extra convert into the out tile below instead.
                pass

            out_t = opool.tile([128, L], F16, tag="out")
            sc_t = scalep_t if (g is None or g > 0) else scalen_t
            bi_t = biasp_t if (g is None or g > 0) else biasn_t
            if path_a:
                if TP_MAJOR:
                    o3 = out_t[:].rearrange("p (t m) -> p t m", t=TP)
                    for tp in range(TP):
                        j = st * TP + tp
                        nc.scalar.activation(
                            o3[:, tp, :], s3[:, tp, :],
                            mybir.ActivationFunctionType.Identity,
                            bias=bi_t[:, j:j + 1], scale=sc_t[:, j:j + 1])
                else:
                    o3 = out_t[:].rearrange("p (m t) -> p m t", t=TP)
                    for tp in range(TP):
                        j = st * TP + tp
                        nc.scalar.activation(
                            o3[:, :, tp], s3[:, :, tp],
                            mybir.ActivationFunctionType.Identity,
                            bias=bi_t[:, j:j + 1], scale=sc_t[:, j:j + 1])
            else:
                # path B: convert u16 -> fp16 value, fold sign, then ACT
                ov = out_t[:].bitcast(F16)
                nc.vector.tensor_copy(ov, fm_t[:])
                s3f = ov.rearrange("p (m t) -> p m t", t=TP)
                sgn_b = (sgn_t[:, c0:c1].unsqueeze(2)
                         .to_broadcast((128, mw, TP)))
                nc.vector.scalar_tensor_tensor(
                    s3f, s3f, 30.0, sgn_b,
                    mybir.AluOpType.subtract, mybir.AluOpType.mult)
                o3 = out_t[:].rearrange("p (m t) -> p m t", t=TP)
                for tp in range(TP):
                    j = st * TP + tp
                    nc.scalar.activation(
                        o3[:, :, tp], o3[:, :, tp],
                        mybir.ActivationFunctionType.Identity,
                        bias=bi_t[:, j:j + 1], scale=sc_t[:, j:j + 1])
            nc.sync.dma_start(out_h.ap()[st, :, c0 * TP:c1 * TP], out_t[:])

        for _rep in range(reps):
            for st in range(NST):
                for (c0, c1, g) in chunks:
                    one_chunk(st, c0, c1, g)

    nc.compile()
    return nc


def _in_maps(prep, path_a):
    """Per-core input dicts."""
    fm_dev, base_pair = prep["fm_dev"], prep["base_pair"]
    bias, scale_p = prep["bias"], prep["scale_p"]
    maps = []
    for c in range(NCORE):
        t0 = c * TPC
        fm = fm_dev[t0:t0 + TPC]                    # [TPC, MD, TP]
        if TP_MAJOR:
            fm = fm.transpose(0, 2, 1)              # [TPC, TP, MD]
        d = {
            "fm": np.ascontiguousarray(fm).reshape(TPC, FWD),
            "basep": base_pair[t0:t0 + TPC].reshape(NST, 128, NPAIR),
            "scalep": scale_p[t0:t0 + TPC].reshape(NST, 128, TP),
        }
        if path_a:
            d["scalen"] = -d["scalep"]
            d["biasp"] = (bias[t0:t0 + TPC].reshape(NST, 128, TP)
                          - np.float32(30.0) * d["scalep"])
            d["biasn"] = (bias[t0:t0 + TPC].reshape(NST, 128, TP)
                          - np.float32(30.0) * d["scalen"])
        else:
            # device computes x = sgn*(e-15); out = scale_p*x + bias
            d["biasp"] = bias[t0:t0 + TPC].reshape(NST, 128, TP)
            d["sgn"] = prep["sgn_m"].astype(np.float16)
        maps.append(d)
    return maps


def _patch_and_assemble(dev_f32, prep):
    """dev_f32: [NTOK, T2, MD, TP] f32 device result (fp16-upcast).
    Patch fp16-rounding boundary cases + sign exceptions exactly, insert
    the ori column, scatter device columns to perm positions."""
    f32 = np.float32
    ddiff, e_true = _ddiff_lut()
    X = prep["base"][:, :, None, :] ^ prep["fm_dev"][:, None, :, :]
    bad = np.nonzero(ddiff[X])
    if bad[0].size:
        xt, t2, jc, tp = bad
        et = e_true[X[bad]].astype(np.float32)
        s = (et + f32(1.0)) / f32(H)
        dis = (prep["sgn_m"][jc] * prep["ps_tok"][xt, t2, tp]
               * (f32(1.0) - s) * prep["valn"][xt, t2])
        dev_f32[bad] = (prep["A"][xt, t2, tp] + dis) * f32(1.0 / TP)

    # negated candidate that is actually 0: sign is +1, X = posmag
    for tok, jc, tp in prep["exc"]:
        pm = prep["pm_tok"][tok, :, tp]            # [T2]
        ps = prep["ps_tok"][tok, :, tp]
        s0 = (_exp_log2p1(pm) + 1).astype(np.float32) / f32(H)
        dis_cnc = ps * (f32(1.0) - s0) * prep["valn"][tok]
        dev_f32[tok, :, jc, tp] = (prep["A"][tok, :, tp] + dis_cnc) * f32(1.0 / TP)

    out = np.empty((NTOK, T2, M, TP), np.float32)
    jo = prep["jo"]
    out[:, :, :jo, :] = dev_f32[:, :, :jo, :]
    out[:, :, jo + 1:, :] = dev_f32[:, :, jo:, :]
    # ori column: candidate = ori (sign +1), X = base
    e0 = _exp_log2p1(prep["base"].astype(np.int64))
    s0 = (e0 + 1).astype(np.float32) / f32(H)
    dis0 = prep["ps_tok"] * (f32(1.0) - s0) * prep["valn"][:, :, None]
    out[:, :, jo, :] = (prep["A"] + dis0) * f32(1.0 / TP)
    return out


def kernel(sta_loc, pos_loc, val_n, rand_raw, perm, _sim=False):
    prep = _host_prep(sta_loc, pos_loc, val_n, rand_raw, perm)
    chunks, path_a = _chunks_for(prep["sgn_m"])
    nc = _build_program(chunks, path_a)
    maps = _in_maps(prep, path_a)

    if _sim:
        from concourse.bass_interp import CoreSim
        results = []
        for c in range(NCORE):
            sim = CoreSim(nc, trace=False)
            for k, v in maps[c].items():
                sim.tensor(k)[:] = v
            sim.simulate(check_with_hw=False)
            results.append({"out": np.array(sim.tensor("out"))})
    else:
        from concourse.bass_utils import run_bass_kernel_spmd
        res = run_bass_kernel_spmd(nc, maps, list(range(NCORE)))
        results = res.results

    dev = np.empty((NTOK, T2, MD, TP), np.float32)
    for c in range(NCORE):
        if TP_MAJOR:
            o = np.asarray(results[c]["out"]).reshape(NST, TOK_ST, T2, TP, MD)
            o = o.transpose(0, 1, 2, 4, 3)
        else:
            o = np.asarray(results[c]["out"]).reshape(NST, TOK_ST, T2, MD, TP)
        for st in range(NST):
            tok0 = c * TPC + st * TOK_ST
            dev[tok0:tok0 + TOK_ST] = o[st].astype(np.float32)
    out = _patch_and_assemble(dev, prep)
    return out.reshape(B, T1, T2, M, TP)


if __name__ == "__main__":
    pass
